# revision 13
# baseline (speedup 1.0000x reference)
"""Trainium2 Bass kernel for nn_CrossModalAttention (B=16384, GNN=512, TR=768, F=1024).

Math (seq_len==1 degenerate attention, see reference):
    gp = g @ Wg.T + bg                       [B, F]
    tp = t @ Wt.T + bt                       [B, F]
    ga = (tp @ Wv.T + bv) @ Wo.T + bo        (attention(g, t, t))
    ta = (gp @ Wv.T + bv) @ Wo.T + bo
    h  = gelu([ga, ta] @ W1.T + b1)
    out = h @ W2.T + b2 + gp + tp

The whole affine attention+fusion prefix folds down to the raw inputs
on the host:
    P1 = W1[:, :F] @ Wo @ Wv @ Wt            [F, TR]   (multiplies t)
    P2 = W1[:, F:] @ Wo @ Wv @ Wg            [F, GNN]  (multiplies g)
    cv = (W1[:,:F]+W1[:,F:]) @ (Wo@bv+bo) + b1 + P-folded bias terms
    h  = gelu(P1 @ t.T + P2 @ g.T + cv)               [F, B] transposed
    out = W2 @ h + Wg @ g.T + Wt @ t.T + (bg+bt+b2)

Device kernel (transposed [feature, batch] layout, data parallel over 8
cores, 2048 batch rows each, 4 column blocks of 512). Every matmul is
fp8e4 DoubleRow (two K=128 slabs per pass at 0.5 cycles/row):
  C phase: h = gelu(P12 @ [t;g] / sp + cv); P12 absmax-prescaled by a
           power of two on host, descaled inside the Gelu activation.
  D phase: one PSUM accumulation per output tile:
             s2*(W2@h + Wg@g + Wt@t)
           where the input projections use an error-compensated hi/lo
           fp8 split (x = hi(x) + lo(x), W*s2 = hi + lo, dropping only
           the tiny lo*lo term) at a single shared scale s2, so a
           single epilogue activation descales and adds the bias.
           gp/tp are never materialized; bf16 is never needed.
fp8 tensors travel as uint8 DRAM/SBUF and are bitcast to float8e4 at
the matmul/activation, so the host<->device path never sees fp8 dtypes.
"""

import sys

import numpy as np

for _p in ("/opt/trn_rl_repo", "/root/.axon_site/_ro/trn_rl_repo"):
    if _p not in sys.path:
        sys.path.append(_p)

import ml_dtypes

import concourse.bass as bass
import concourse.mybir as mybir
import concourse.tile as tile
from concourse.bass import ts
from concourse.bass_utils import run_bass_kernel_spmd

B = 16384
GNN = 512
TR = 768
F = 1024
N_CORES = 8
B_LOC = B // N_CORES  # 2048
P = 128

KG = GNN // P  # 4
KT = TR // P  # 6
KC = KT + KG  # 10 contraction slabs for the C (fused) stage
KF = F // P  # 8

NB = 512  # batch-column block per step
NBLK = B_LOC // NB
PSUM_BUFS = 8
IO_BUFS = 2
AF = mybir.ActivationFunctionType
DR = mybir.MatmulPerfMode.DoubleRow
FP8 = mybir.dt.float8e4

_DMA_OPCODES = ("DMACopy", "DMATranspose", "EventSemaphore", "TriggeredCopy")


def _legalize_waits(bir: dict) -> dict:
    """Walrus on this stack accepts only ONE sync-wait per engine instruction
    ("Too many sync wait commands"). Hoist extra waits onto standalone
    EventSemaphore ops (what nc.<engine>.wait_ge emits) on the same engine."""
    ctr = 0

    def hoist(out, inst, w):
        nonlocal ctr
        ctr += 1
        out.append(
            {
                "debug": inst.get("debug", 0),
                "engine": inst["engine"],
                "ins": [],
                "outs": [],
                "name": f"I-lgw-{ctr}",
                "opcode": "EventSemaphore",
                "sync_info": {"on_update": [], "on_wait": [w]},
            }
        )

    for fn in bir["functions"]:
        for blk in fn["blocks"]:
            out = []
            for inst in blk["instructions"]:
                si = inst.get("sync_info")
                waits = (si.get("on_wait") or []) if si else []
                op = inst.get("opcode")
                if op == "EventSemaphore":
                    pass
                elif op in ("DMACopy", "DMATranspose", "TriggeredCopy"):
                    # keep one wait (prefer a queue DMA* sem) on the descriptor,
                    # hoist the rest onto the issuing sequencer
                    if len(waits) > 1:
                        keep = [w for w in waits if w["ant_name"].startswith("DMA")]
                        drop = [w for w in waits if not w["ant_name"].startswith("DMA")]
                        if not keep:
                            keep = [waits[-1]]
                            drop = waits[:-1]
                        while len(keep) > 1:
                            drop.append(keep.pop(0))
                        for w in drop:
                            hoist(out, inst, w)
                        si["on_wait"] = keep
                elif len(waits) > 1:
                    for w in waits[:-1]:
                        hoist(out, inst, w)
                    si["on_wait"] = waits[-1:]
                out.append(inst)
            blk["instructions"] = out
    return bir


def _attach_wait_legalizer(nc):
    import json as _json

    orig_fn = nc.to_json_bytes

    def _patched():
        bir = _json.loads(orig_fn())
        _legalize_waits(bir)
        return _json.dumps(bir).encode()

    nc.to_json_bytes = _patched


def build_module(sp_inv=1.0 / 4096, s2_inv=1.0 / 2048, repeat=1):
    nc = bass.Bass()
    f32 = mybir.dt.float32
    u8 = mybir.dt.uint8

    g8T = nc.dram_tensor("g8T", [GNN, B_LOC], u8, kind="ExternalInput")
    t8T = nc.dram_tensor("t8T", [TR, B_LOC], u8, kind="ExternalInput")
    glT = nc.dram_tensor("glT", [GNN, B_LOC], u8, kind="ExternalInput")
    tlT = nc.dram_tensor("tlT", [TR, B_LOC], u8, kind="ExternalInput")
    p12T = nc.dram_tensor("p12T", [KC * P, F], u8, kind="ExternalInput")
    wghT = nc.dram_tensor("wghT", [GNN, F], u8, kind="ExternalInput")
    wglT = nc.dram_tensor("wglT", [GNN, F], u8, kind="ExternalInput")
    wthT = nc.dram_tensor("wthT", [TR, F], u8, kind="ExternalInput")
    wtlT = nc.dram_tensor("wtlT", [TR, F], u8, kind="ExternalInput")
    w2T = nc.dram_tensor("w2T", [F, F], u8, kind="ExternalInput")
    cv = nc.dram_tensor("cv", [F], f32, kind="ExternalInput")
    bsum = nc.dram_tensor("bsum", [F], f32, kind="ExternalInput")
    outT = nc.dram_tensor("outT", [F, B_LOC], f32, kind="ExternalOutput")

    g8_ap = g8T[:].rearrange("(k p) b -> p k b", p=P)
    t8_ap = t8T[:].rearrange("(k p) b -> p k b", p=P)
    gl_ap = glT[:].rearrange("(k p) b -> p k b", p=P)
    tl_ap = tlT[:].rearrange("(k p) b -> p k b", p=P)
    p12_ap = p12T[:].rearrange("(k p) f -> p k f", p=P)
    out_ap = outT[:].rearrange("(k p) b -> p k b", p=P)

    def wf(dram):  # weight DRAM -> [P, k, F] view
        return dram[:].rearrange("(k p) f -> p k f", p=P)

    with tile.TileContext(nc) as tc:
        with (
            tc.tile_pool(name="const", bufs=1) as const,
            tc.tile_pool(name="io", bufs=IO_BUFS) as io,
            tc.tile_pool(name="hbuf", bufs=2) as hbuf,
            tc.tile_pool(name="psum", bufs=PSUM_BUFS, space="PSUM") as psum,
        ):
            p12 = const.tile([P, KC, F], u8)
            cv_t = const.tile([P, KF], f32)
            wgh = const.tile([P, KG, F], u8)
            wgl = const.tile([P, KG, F], u8)
            wth = const.tile([P, KT, F], u8)
            wtl = const.tile([P, KT, F], u8)
            w2 = const.tile([P, KF, F], u8)
            bs_t = const.tile([P, KF], f32)

            # p12 in two column halves: C_0 needs only the first half, the
            # second arrives while C_0..C_3 run
            half = (KF // 2) * P
            nc.sync.dma_start(out=p12[:, :, 0:half], in_=p12_ap[:, :, 0:half])

            for blk in [b for _ in range(repeat) for b in range(NBLK)]:
                bs = slice(blk * NB, (blk + 1) * NB)
                t8 = io.tile([P, KT, NB], u8, tag="t8")
                nc.sync.dma_start(out=t8, in_=t8_ap[:, :, bs])
                g8 = io.tile([P, KG, NB], u8, tag="g8")
                nc.sync.dma_start(out=g8, in_=g8_ap[:, :, bs])
                gl = io.tile([P, KG, NB], u8, tag="gl")
                tl = io.tile([P, KT, NB], u8, tag="tl")
                if blk == 0:
                    # stream in D-sub-phase consumption order: the whole first
                    # block stays DMA-fed with no PE stall
                    nc.sync.dma_start(out=p12[:, :, half:F], in_=p12_ap[:, :, half:F])
                    nc.sync.dma_start(out=cv_t, in_=cv[:].rearrange("(k p) -> p k", p=P))
                    nc.sync.dma_start(out=wth, in_=wf(wthT))
                    nc.sync.dma_start(out=wgh, in_=wf(wghT))
                    nc.sync.dma_start(out=tl, in_=tl_ap[:, :, bs])
                    nc.sync.dma_start(out=gl, in_=gl_ap[:, :, bs])
                    nc.sync.dma_start(out=wtl, in_=wf(wtlT))
                    nc.sync.dma_start(out=wgl, in_=wf(wglT))
                    nc.sync.dma_start(out=bs_t, in_=bsum[:].rearrange("(k p) -> p k", p=P))
                    nc.sync.dma_start(out=w2, in_=wf(w2T))
                else:
                    nc.sync.dma_start(out=gl, in_=gl_ap[:, :, bs])
                    nc.sync.dma_start(out=tl, in_=tl_ap[:, :, bs])

                h8 = hbuf.tile([P, KF, NB], u8, tag="h8")
                out_t = io.tile([P, KF, NB], f32, tag="out_t")

                def dr(ps, w_t, kofs, j, x_t, kk, start, stop):
                    nc.tensor.matmul(
                        ps,
                        w_t[:, kofs + kk : kofs + kk + 2, ts(j, P)].bitcast(FP8),
                        x_t[:, kk : kk + 2, :].bitcast(FP8),
                        start=start,
                        stop=stop,
                        perf_mode=DR,
                    )

                # C: h = gelu((P1@t + P2@g) * sp_inv + cv)
                # two sub-phases across all 8 PSUM banks: all t-passes first
                # (t8 lands before g8), so the g8 DMA never stalls the PE
                c_ps = []
                for j in range(KF):
                    ps = psum.tile([P, NB], f32, tag="ps")
                    c_ps.append(ps)
                    for kk in range(0, KT, 2):
                        dr(ps, p12, 0, j, t8, kk, kk == 0, False)
                for j in range(KF):
                    ps = c_ps[j]
                    for kk in range(0, KG, 2):
                        dr(ps, p12, KT, j, g8, kk, False, kk + 2 >= KG)
                    nc.scalar.activation(
                        h8[:, j, :].bitcast(FP8), ps, AF.Gelu, bias=cv_t[:, j : j + 1], scale=sp_inv
                    )

                # D: out = (W2@h + Wg@g + Wt@t) * s2_inv + bsum
                # input projections via hi/lo fp8: hi@hi + hi@lo + lo@hi, all
                # at scale s2 in one PSUM group per tile. Sub-phase-major
                # across all 8 banks, ordered to match DMA arrival: hi@hi,
                # then hi@lo (needs gl/tl), then lo@hi (needs wgl/wtl), then
                # W2@h (needs h8) with the epilogue interleaved per tile.
                d_ps = []
                for _j in range(KF):
                    ps = psum.tile([P, NB], f32, tag="ps")
                    d_ps.append(ps)
                for j in range(KF):
                    for kk in range(0, KT, 2):
                        dr(d_ps[j], wth, 0, j, t8, kk, kk == 0, False)
                    for kk in range(0, KG, 2):
                        dr(d_ps[j], wgh, 0, j, g8, kk, False, False)
                for j in range(KF):
                    for kk in range(0, KT, 2):
                        dr(d_ps[j], wth, 0, j, tl, kk, False, False)
                    for kk in range(0, KG, 2):
                        dr(d_ps[j], wgh, 0, j, gl, kk, False, False)
                for j in range(KF):
                    for kk in range(0, KT, 2):
                        dr(d_ps[j], wtl, 0, j, t8, kk, False, False)
                    for kk in range(0, KG, 2):
                        dr(d_ps[j], wgl, 0, j, g8, kk, False, False)
                for j in range(KF):
                    ps = d_ps[j]
                    for kk in range(0, KF, 2):
                        dr(ps, w2, 0, j, h8, kk, False, kk + 2 >= KF)
                    nc.scalar.activation(
                        out_t[:, j, :], ps, AF.Identity, bias=bs_t[:, j : j + 1], scale=s2_inv
                    )
                    # out DMA on the Activation HWDGE queue: keeps the SP
                    # input queue free of head-of-line blocking. Last block
                    # flushes per-j so the tail after the final matmul is one
                    # small chunk.
                    if blk == NBLK - 1:
                        nc.scalar.dma_start(out=out_ap[:, j : j + 1, bs], in_=out_t[:, j : j + 1, :])
                    elif j % 2 == 1:
                        nc.scalar.dma_start(
                            out=out_ap[:, j - 1 : j + 1, bs], in_=out_t[:, j - 1 : j + 1, :]
                        )

    _attach_wait_legalizer(nc)
    return nc


def _pow2_scale(m, target=224.0):
    if m == 0.0 or not np.isfinite(m):
        return 1.0
    return float(2.0 ** np.floor(np.log2(target / m)))


def prepare_inputs(gnn_features, transformer_features, Wg, bg, Wt, bt, Wv, bv, Wo, bo, W1, b1, W2, b2):
    """Host-side: fold attention+fusion prefix down to the raw inputs,
    quantize everything to fp8e4 (hi/lo for the input projections),
    transpose to [feature, batch]."""
    f64 = np.float64
    fp8 = ml_dtypes.float8_e4m3

    A = Wo.astype(f64) @ Wv.astype(f64)
    W1a = W1[:, :F].astype(f64)
    W1b = W1[:, F:].astype(f64)
    M1 = W1a @ A
    M2 = W1b @ A
    c = (W1a + W1b) @ (Wo.astype(f64) @ bv.astype(f64) + bo.astype(f64)) + b1.astype(f64)
    P1 = M1 @ Wt.astype(f64)  # [F, TR]
    P2 = M2 @ Wg.astype(f64)  # [F, GNN]
    cvec = c + M1 @ bt.astype(f64) + M2 @ bg.astype(f64)

    p12 = np.concatenate([P1.T, P2.T], axis=0)  # [TR+GNN, F] rows=contraction
    sp = _pow2_scale(float(np.abs(p12).max()))
    s2 = _pow2_scale(max(float(np.abs(w).max()) for w in (W2, Wg, Wt)))

    def hilo(x):
        hi = x.astype(fp8)
        lo = (x - hi.astype(f64)).astype(fp8)
        return hi.view(np.uint8), lo.view(np.uint8)

    p12T = np.ascontiguousarray(p12 * sp).astype(fp8).view(np.uint8)
    w2T = np.ascontiguousarray(W2.T.astype(f64) * s2).astype(fp8).view(np.uint8)
    wghT, wglT = hilo(np.ascontiguousarray(Wg.T.astype(f64) * s2))
    wthT, wtlT = hilo(np.ascontiguousarray(Wt.T.astype(f64) * s2))
    bsum = (bg.astype(f64) + bt.astype(f64) + b2.astype(f64)).astype(np.float32)

    shared = {
        "p12T": p12T,
        "wghT": wghT,
        "wglT": wglT,
        "wthT": wthT,
        "wtlT": wtlT,
        "w2T": w2T,
        "cv": cvec.astype(np.float32),
        "bsum": bsum,
    }
    in_maps = []
    for i in range(N_CORES):
        rows = slice(i * B_LOC, (i + 1) * B_LOC)
        g8, gl = hilo(np.ascontiguousarray(gnn_features[rows].T).astype(f64))
        t8, tl = hilo(np.ascontiguousarray(transformer_features[rows].T).astype(f64))
        in_maps.append({"g8T": g8, "glT": gl, "t8T": t8, "tlT": tl, **shared})
    return in_maps, 1.0 / sp, 1.0 / s2


def run(inputs, trace=False, **kw):
    in_maps, sp_inv, s2_inv = prepare_inputs(**inputs)
    nc = build_module(sp_inv=sp_inv, s2_inv=s2_inv)
    res = run_bass_kernel_spmd(nc, in_maps, core_ids=list(range(N_CORES)), trace=trace, **kw)
    out = np.concatenate([r["outT"].T for r in res.results], axis=0).astype(np.float32)
    return out, res


def kernel(**inputs) -> np.ndarray:
    out, _ = run(inputs, trace=False)
    return out


# revision 14
# speedup vs baseline: 1.0450x; 1.0450x over previous
"""Trainium2 Bass kernel for nn_CrossModalAttention (B=16384, GNN=512, TR=768, F=1024).

Math (seq_len==1 degenerate attention, see reference):
    gp = g @ Wg.T + bg                       [B, F]
    tp = t @ Wt.T + bt                       [B, F]
    ga = (tp @ Wv.T + bv) @ Wo.T + bo        (attention(g, t, t))
    ta = (gp @ Wv.T + bv) @ Wo.T + bo
    h  = gelu([ga, ta] @ W1.T + b1)
    out = h @ W2.T + b2 + gp + tp

The whole affine attention+fusion prefix folds down to the raw inputs
on the host:
    P1 = W1[:, :F] @ Wo @ Wv @ Wt            [F, TR]   (multiplies t)
    P2 = W1[:, F:] @ Wo @ Wv @ Wg            [F, GNN]  (multiplies g)
    cv = (W1[:,:F]+W1[:,F:]) @ (Wo@bv+bo) + b1 + P-folded bias terms
    h  = gelu(P1 @ t.T + P2 @ g.T + cv)               [F, B] transposed
    out = W2 @ h + Wg @ g.T + Wt @ t.T + (bg+bt+b2)

Device kernel (transposed [feature, batch] layout, data parallel over 8
cores, 2048 batch rows each, 4 column blocks of 512). Every matmul is
fp8e4 DoubleRow (two K=128 slabs per pass at 0.5 cycles/row):
  C phase: h = gelu(P12 @ [t;g] / sp + cv); P12 absmax-prescaled by a
           power of two on host, descaled inside the Gelu activation.
  D phase: one PSUM accumulation per output tile:
             s2*(W2@h + Wg@g + Wt@t)
           where the input projections use an error-compensated hi/lo
           fp8 split (x = hi(x) + lo(x), W*s2 = hi + lo, dropping only
           the tiny lo*lo term) at a single shared scale s2, so a
           single epilogue activation descales and adds the bias.
           gp/tp are never materialized; bf16 is never needed.
fp8 tensors travel as uint8 DRAM/SBUF and are bitcast to float8e4 at
the matmul/activation, so the host<->device path never sees fp8 dtypes.
"""

import sys

import numpy as np

for _p in ("/opt/trn_rl_repo", "/root/.axon_site/_ro/trn_rl_repo"):
    if _p not in sys.path:
        sys.path.append(_p)

import ml_dtypes

import concourse.bass as bass
import concourse.mybir as mybir
import concourse.tile as tile
from concourse.bass import ts
from concourse.bass_utils import run_bass_kernel_spmd

B = 16384
GNN = 512
TR = 768
F = 1024
N_CORES = 8
B_LOC = B // N_CORES  # 2048
P = 128

KG = GNN // P  # 4
KT = TR // P  # 6
KC = KT + KG  # 10 contraction slabs for the C (fused) stage
KF = F // P  # 8

NB = 512  # batch-column block per step
NBLK = B_LOC // NB
PSUM_BUFS = 8
IO_BUFS = 2
AF = mybir.ActivationFunctionType
DR = mybir.MatmulPerfMode.DoubleRow
FP8 = mybir.dt.float8e4

_DMA_OPCODES = ("DMACopy", "DMATranspose", "EventSemaphore", "TriggeredCopy")


def _legalize_waits(bir: dict) -> dict:
    """Walrus on this stack accepts only ONE sync-wait per engine instruction
    ("Too many sync wait commands"). Hoist extra waits onto standalone
    EventSemaphore ops (what nc.<engine>.wait_ge emits) on the same engine."""
    ctr = 0

    def hoist(out, inst, w):
        nonlocal ctr
        ctr += 1
        out.append(
            {
                "debug": inst.get("debug", 0),
                "engine": inst["engine"],
                "ins": [],
                "outs": [],
                "name": f"I-lgw-{ctr}",
                "opcode": "EventSemaphore",
                "sync_info": {"on_update": [], "on_wait": [w]},
            }
        )

    for fn in bir["functions"]:
        for blk in fn["blocks"]:
            out = []
            for inst in blk["instructions"]:
                si = inst.get("sync_info")
                waits = (si.get("on_wait") or []) if si else []
                op = inst.get("opcode")
                if op == "EventSemaphore":
                    pass
                elif op in ("DMACopy", "DMATranspose", "TriggeredCopy"):
                    # keep one wait (prefer a queue DMA* sem) on the descriptor,
                    # hoist the rest onto the issuing sequencer
                    if len(waits) > 1:
                        keep = [w for w in waits if w["ant_name"].startswith("DMA")]
                        drop = [w for w in waits if not w["ant_name"].startswith("DMA")]
                        if not keep:
                            keep = [waits[-1]]
                            drop = waits[:-1]
                        while len(keep) > 1:
                            drop.append(keep.pop(0))
                        for w in drop:
                            hoist(out, inst, w)
                        si["on_wait"] = keep
                elif len(waits) > 1:
                    for w in waits[:-1]:
                        hoist(out, inst, w)
                    si["on_wait"] = waits[-1:]
                out.append(inst)
            blk["instructions"] = out
    return bir


def _attach_wait_legalizer(nc):
    import json as _json

    orig_fn = nc.to_json_bytes

    def _patched():
        bir = _json.loads(orig_fn())
        _legalize_waits(bir)
        return _json.dumps(bir).encode()

    nc.to_json_bytes = _patched


def build_module(sp_inv=1.0 / 4096, s2_inv=1.0 / 2048, repeat=1):
    nc = bass.Bass()
    f32 = mybir.dt.float32
    u8 = mybir.dt.uint8

    g8T = nc.dram_tensor("g8T", [GNN, B_LOC], u8, kind="ExternalInput")
    t8T = nc.dram_tensor("t8T", [TR, B_LOC], u8, kind="ExternalInput")
    glT = nc.dram_tensor("glT", [GNN, B_LOC], u8, kind="ExternalInput")
    tlT = nc.dram_tensor("tlT", [TR, B_LOC], u8, kind="ExternalInput")
    p12T = nc.dram_tensor("p12T", [KC * P, F], u8, kind="ExternalInput")
    wghT = nc.dram_tensor("wghT", [GNN, F], u8, kind="ExternalInput")
    wglT = nc.dram_tensor("wglT", [GNN, F], u8, kind="ExternalInput")
    wthT = nc.dram_tensor("wthT", [TR, F], u8, kind="ExternalInput")
    wtlT = nc.dram_tensor("wtlT", [TR, F], u8, kind="ExternalInput")
    w2T = nc.dram_tensor("w2T", [F, F], u8, kind="ExternalInput")
    cv = nc.dram_tensor("cv", [F], f32, kind="ExternalInput")
    bsum = nc.dram_tensor("bsum", [F], f32, kind="ExternalInput")
    outT = nc.dram_tensor("outT", [F, B_LOC], f32, kind="ExternalOutput")

    g8_ap = g8T[:].rearrange("(k p) b -> p k b", p=P)
    t8_ap = t8T[:].rearrange("(k p) b -> p k b", p=P)
    gl_ap = glT[:].rearrange("(k p) b -> p k b", p=P)
    tl_ap = tlT[:].rearrange("(k p) b -> p k b", p=P)
    p12_ap = p12T[:].rearrange("(k p) f -> p k f", p=P)
    out_ap = outT[:].rearrange("(k p) b -> p k b", p=P)

    def wf(dram):  # weight DRAM -> [P, k, F] view
        return dram[:].rearrange("(k p) f -> p k f", p=P)

    with tile.TileContext(nc) as tc:
        with (
            tc.tile_pool(name="const", bufs=1) as const,
            tc.tile_pool(name="io", bufs=IO_BUFS) as io,
            tc.tile_pool(name="hbuf", bufs=2) as hbuf,
            tc.tile_pool(name="psum", bufs=PSUM_BUFS, space="PSUM") as psum,
        ):
            p12 = const.tile([P, KC, F], u8)
            cv_t = const.tile([P, KF], f32)
            wgh = const.tile([P, KG, F], u8)
            wgl = const.tile([P, KG, F], u8)
            wth = const.tile([P, KT, F], u8)
            wtl = const.tile([P, KT, F], u8)
            w2 = const.tile([P, KF, F], u8)
            bs_t = const.tile([P, KF], f32)

            # p12 in two column halves: C_0 needs only the first half, the
            # second arrives while C_0..C_3 run
            half = (KF // 2) * P
            nc.sync.dma_start(out=p12[:, :, 0:half], in_=p12_ap[:, :, 0:half])

            for blk in [b for _ in range(repeat) for b in range(NBLK)]:
                bs = slice(blk * NB, (blk + 1) * NB)
                t8 = io.tile([P, KT, NB], u8, tag="t8")
                nc.sync.dma_start(out=t8, in_=t8_ap[:, :, bs])
                g8 = io.tile([P, KG, NB], u8, tag="g8")
                nc.sync.dma_start(out=g8, in_=g8_ap[:, :, bs])
                gl = io.tile([P, KG, NB], u8, tag="gl")
                tl = io.tile([P, KT, NB], u8, tag="tl")
                if blk == 0:
                    # stream in D-sub-phase consumption order: the whole first
                    # block stays DMA-fed with no PE stall
                    nc.sync.dma_start(out=p12[:, :, half:F], in_=p12_ap[:, :, half:F])
                    nc.sync.dma_start(out=cv_t, in_=cv[:].rearrange("(k p) -> p k", p=P))
                    nc.sync.dma_start(out=wth, in_=wf(wthT))
                    nc.sync.dma_start(out=wgh, in_=wf(wghT))
                    nc.sync.dma_start(out=tl, in_=tl_ap[:, :, bs])
                    nc.sync.dma_start(out=gl, in_=gl_ap[:, :, bs])
                    nc.sync.dma_start(out=wtl, in_=wf(wtlT))
                    nc.sync.dma_start(out=wgl, in_=wf(wglT))
                    nc.sync.dma_start(out=bs_t, in_=bsum[:].rearrange("(k p) -> p k", p=P))
                    nc.sync.dma_start(out=w2, in_=wf(w2T))
                else:
                    nc.sync.dma_start(out=gl, in_=gl_ap[:, :, bs])
                    nc.sync.dma_start(out=tl, in_=tl_ap[:, :, bs])

                h8 = hbuf.tile([P, KF, NB], u8, tag="h8")
                out_t = io.tile([P, KF, NB], f32, tag="out_t")

                def dr(ps, w_t, kofs, j, x_t, kk, start, stop):
                    nc.tensor.matmul(
                        ps,
                        w_t[:, kofs + kk : kofs + kk + 2, ts(j, P)].bitcast(FP8),
                        x_t[:, kk : kk + 2, :].bitcast(FP8),
                        start=start,
                        stop=stop,
                        perf_mode=DR,
                    )

                # C: h = gelu((P1@t + P2@g) * sp_inv + cv)
                # two sub-phases across all 8 PSUM banks: all t-passes first
                # (t8 lands before g8), so the g8 DMA never stalls the PE
                c_ps = []
                for j in range(KF):
                    ps = psum.tile([P, NB], f32, tag="ps")
                    c_ps.append(ps)
                    for kk in range(0, KT, 2):
                        dr(ps, p12, 0, j, t8, kk, kk == 0, False)
                for j in range(KF):
                    ps = c_ps[j]
                    for kk in range(0, KG, 2):
                        dr(ps, p12, KT, j, g8, kk, False, kk + 2 >= KG)
                    nc.scalar.activation(
                        h8[:, j, :].bitcast(FP8), ps, AF.Gelu, bias=cv_t[:, j : j + 1], scale=sp_inv
                    )

                # D: out = (W2@h + Wg@g + Wt@t) * s2_inv + bsum
                # input projections via hi/lo fp8: hi@hi + hi@lo + lo@hi, all
                # at scale s2 in one PSUM group per tile. Sub-phase-major
                # across all 8 banks, ordered to match DMA arrival: hi@hi,
                # then hi@lo (needs gl/tl), then lo@hi (needs wgl/wtl), then
                # W2@h (needs h8) with the epilogue interleaved per tile.
                def d_epilogue(ps, j):
                    nc.scalar.activation(
                        out_t[:, j, :], ps, AF.Identity, bias=bs_t[:, j : j + 1], scale=s2_inv
                    )
                    # out DMA on the Activation HWDGE queue: keeps the SP
                    # input queue free of head-of-line blocking
                    if blk == NBLK - 1:
                        nc.scalar.dma_start(out=out_ap[:, j : j + 1, bs], in_=out_t[:, j : j + 1, :])
                    elif j % 2 == 1:
                        nc.scalar.dma_start(
                            out=out_ap[:, j - 1 : j + 1, bs], in_=out_t[:, j - 1 : j + 1, :]
                        )

                if blk < NBLK - 1:
                    # sub-phase-major across all 8 banks, ordered to match DMA
                    # arrival in block 0 (hi@hi, then hi@lo needing gl/tl,
                    # then lo@hi needing wgl/wtl, then W2@h needing h8)
                    d_ps = []
                    for _j in range(KF):
                        ps = psum.tile([P, NB], f32, tag="ps")
                        d_ps.append(ps)
                    for j in range(KF):
                        for kk in range(0, KT, 2):
                            dr(d_ps[j], wth, 0, j, t8, kk, kk == 0, False)
                        for kk in range(0, KG, 2):
                            dr(d_ps[j], wgh, 0, j, g8, kk, False, False)
                    for j in range(KF):
                        for kk in range(0, KT, 2):
                            dr(d_ps[j], wth, 0, j, tl, kk, False, False)
                        for kk in range(0, KG, 2):
                            dr(d_ps[j], wgh, 0, j, gl, kk, False, False)
                    for j in range(KF):
                        for kk in range(0, KT, 2):
                            dr(d_ps[j], wtl, 0, j, t8, kk, False, False)
                        for kk in range(0, KG, 2):
                            dr(d_ps[j], wgl, 0, j, g8, kk, False, False)
                    for j in range(KF):
                        for kk in range(0, KF, 2):
                            dr(d_ps[j], w2, 0, j, h8, kk, False, kk + 2 >= KF)
                        d_epilogue(d_ps[j], j)
                else:
                    # last block: per-tile groups so epilogues + output DMAs
                    # drain progressively and the post-matmul tail is minimal
                    for j in range(KF):
                        ps = psum.tile([P, NB], f32, tag="ps")
                        for kk in range(0, KT, 2):
                            dr(ps, wth, 0, j, t8, kk, kk == 0, False)
                        for kk in range(0, KG, 2):
                            dr(ps, wgh, 0, j, g8, kk, False, False)
                        for kk in range(0, KT, 2):
                            dr(ps, wth, 0, j, tl, kk, False, False)
                        for kk in range(0, KG, 2):
                            dr(ps, wgh, 0, j, gl, kk, False, False)
                        for kk in range(0, KT, 2):
                            dr(ps, wtl, 0, j, t8, kk, False, False)
                        for kk in range(0, KG, 2):
                            dr(ps, wgl, 0, j, g8, kk, False, False)
                        for kk in range(0, KF, 2):
                            dr(ps, w2, 0, j, h8, kk, False, kk + 2 >= KF)
                        d_epilogue(ps, j)

    _attach_wait_legalizer(nc)
    return nc


def _pow2_scale(m, target=224.0):
    if m == 0.0 or not np.isfinite(m):
        return 1.0
    return float(2.0 ** np.floor(np.log2(target / m)))


def prepare_inputs(gnn_features, transformer_features, Wg, bg, Wt, bt, Wv, bv, Wo, bo, W1, b1, W2, b2):
    """Host-side: fold attention+fusion prefix down to the raw inputs,
    quantize everything to fp8e4 (hi/lo for the input projections),
    transpose to [feature, batch]."""
    f64 = np.float64
    fp8 = ml_dtypes.float8_e4m3

    A = Wo.astype(f64) @ Wv.astype(f64)
    W1a = W1[:, :F].astype(f64)
    W1b = W1[:, F:].astype(f64)
    M1 = W1a @ A
    M2 = W1b @ A
    c = (W1a + W1b) @ (Wo.astype(f64) @ bv.astype(f64) + bo.astype(f64)) + b1.astype(f64)
    P1 = M1 @ Wt.astype(f64)  # [F, TR]
    P2 = M2 @ Wg.astype(f64)  # [F, GNN]
    cvec = c + M1 @ bt.astype(f64) + M2 @ bg.astype(f64)

    p12 = np.concatenate([P1.T, P2.T], axis=0)  # [TR+GNN, F] rows=contraction
    sp = _pow2_scale(float(np.abs(p12).max()))
    s2 = _pow2_scale(max(float(np.abs(w).max()) for w in (W2, Wg, Wt)))

    def hilo(x):
        hi = x.astype(fp8)
        lo = (x - hi.astype(f64)).astype(fp8)
        return hi.view(np.uint8), lo.view(np.uint8)

    p12T = np.ascontiguousarray(p12 * sp).astype(fp8).view(np.uint8)
    w2T = np.ascontiguousarray(W2.T.astype(f64) * s2).astype(fp8).view(np.uint8)
    wghT, wglT = hilo(np.ascontiguousarray(Wg.T.astype(f64) * s2))
    wthT, wtlT = hilo(np.ascontiguousarray(Wt.T.astype(f64) * s2))
    bsum = (bg.astype(f64) + bt.astype(f64) + b2.astype(f64)).astype(np.float32)

    shared = {
        "p12T": p12T,
        "wghT": wghT,
        "wglT": wglT,
        "wthT": wthT,
        "wtlT": wtlT,
        "w2T": w2T,
        "cv": cvec.astype(np.float32),
        "bsum": bsum,
    }
    in_maps = []
    for i in range(N_CORES):
        rows = slice(i * B_LOC, (i + 1) * B_LOC)
        g8, gl = hilo(np.ascontiguousarray(gnn_features[rows].T).astype(f64))
        t8, tl = hilo(np.ascontiguousarray(transformer_features[rows].T).astype(f64))
        in_maps.append({"g8T": g8, "glT": gl, "t8T": t8, "tlT": tl, **shared})
    return in_maps, 1.0 / sp, 1.0 / s2


def run(inputs, trace=False, **kw):
    in_maps, sp_inv, s2_inv = prepare_inputs(**inputs)
    nc = build_module(sp_inv=sp_inv, s2_inv=s2_inv)
    res = run_bass_kernel_spmd(nc, in_maps, core_ids=list(range(N_CORES)), trace=trace, **kw)
    out = np.concatenate([r["outT"].T for r in res.results], axis=0).astype(np.float32)
    return out, res


def kernel(**inputs) -> np.ndarray:
    out, _ = run(inputs, trace=False)
    return out


# revision 17
# speedup vs baseline: 1.0484x; 1.0032x over previous
"""Trainium2 Bass kernel for nn_CrossModalAttention (B=16384, GNN=512, TR=768, F=1024).

Math (seq_len==1 degenerate attention, see reference):
    gp = g @ Wg.T + bg                       [B, F]
    tp = t @ Wt.T + bt                       [B, F]
    ga = (tp @ Wv.T + bv) @ Wo.T + bo        (attention(g, t, t))
    ta = (gp @ Wv.T + bv) @ Wo.T + bo
    h  = gelu([ga, ta] @ W1.T + b1)
    out = h @ W2.T + b2 + gp + tp

The whole affine attention+fusion prefix folds down to the raw inputs
on the host:
    P1 = W1[:, :F] @ Wo @ Wv @ Wt            [F, TR]   (multiplies t)
    P2 = W1[:, F:] @ Wo @ Wv @ Wg            [F, GNN]  (multiplies g)
    cv = (W1[:,:F]+W1[:,F:]) @ (Wo@bv+bo) + b1 + P-folded bias terms
    h  = gelu(P1 @ t.T + P2 @ g.T + cv)               [F, B] transposed
    out = W2 @ h + Wg @ g.T + Wt @ t.T + (bg+bt+b2)

Device kernel (transposed [feature, batch] layout, data parallel over 8
cores, 2048 batch rows each, 4 column blocks of 512). Every matmul is
fp8e4 DoubleRow (two K=128 slabs per pass at 0.5 cycles/row):
  C phase: h = gelu(P12 @ [t;g] / sp + cv); P12 absmax-prescaled by a
           power of two on host, descaled inside the Gelu activation.
  D phase: one PSUM accumulation per output tile:
             s2*(W2@h + Wg@g + Wt@t)
           where the input projections use an error-compensated hi/lo
           fp8 split (x = hi(x) + lo(x), W*s2 = hi + lo, dropping only
           the tiny lo*lo term) at a single shared scale s2, so a
           single epilogue activation descales and adds the bias.
           gp/tp are never materialized; bf16 is never needed.
fp8 tensors travel as uint8 DRAM/SBUF and are bitcast to float8e4 at
the matmul/activation, so the host<->device path never sees fp8 dtypes.
"""

import sys

import numpy as np

for _p in ("/opt/trn_rl_repo", "/root/.axon_site/_ro/trn_rl_repo"):
    if _p not in sys.path:
        sys.path.append(_p)

import ml_dtypes

import concourse.bass as bass
import concourse.mybir as mybir
import concourse.tile as tile
from concourse.bass import ts
from concourse.bass_utils import run_bass_kernel_spmd

B = 16384
GNN = 512
TR = 768
F = 1024
N_CORES = 8
B_LOC = B // N_CORES  # 2048
P = 128

KG = GNN // P  # 4
KT = TR // P  # 6
KC = KT + KG  # 10 contraction slabs for the C (fused) stage
KF = F // P  # 8

NB = 512  # batch-column block per step
NBLK = B_LOC // NB
PSUM_BUFS = 8
IO_BUFS = 2
AF = mybir.ActivationFunctionType
DR = mybir.MatmulPerfMode.DoubleRow
FP8 = mybir.dt.float8e4

_DMA_OPCODES = ("DMACopy", "DMATranspose", "EventSemaphore", "TriggeredCopy")


def _legalize_waits(bir: dict) -> dict:
    """Walrus on this stack accepts only ONE sync-wait per engine instruction
    ("Too many sync wait commands"). Hoist extra waits onto standalone
    EventSemaphore ops (what nc.<engine>.wait_ge emits) on the same engine."""
    ctr = 0

    def hoist(out, inst, w):
        nonlocal ctr
        ctr += 1
        out.append(
            {
                "debug": inst.get("debug", 0),
                "engine": inst["engine"],
                "ins": [],
                "outs": [],
                "name": f"I-lgw-{ctr}",
                "opcode": "EventSemaphore",
                "sync_info": {"on_update": [], "on_wait": [w]},
            }
        )

    for fn in bir["functions"]:
        for blk in fn["blocks"]:
            out = []
            for inst in blk["instructions"]:
                si = inst.get("sync_info")
                waits = (si.get("on_wait") or []) if si else []
                op = inst.get("opcode")
                if op == "EventSemaphore":
                    pass
                elif op in ("DMACopy", "DMATranspose", "TriggeredCopy"):
                    # keep one wait (prefer a queue DMA* sem) on the descriptor,
                    # hoist the rest onto the issuing sequencer
                    if len(waits) > 1:
                        keep = [w for w in waits if w["ant_name"].startswith("DMA")]
                        drop = [w for w in waits if not w["ant_name"].startswith("DMA")]
                        if not keep:
                            keep = [waits[-1]]
                            drop = waits[:-1]
                        while len(keep) > 1:
                            drop.append(keep.pop(0))
                        for w in drop:
                            hoist(out, inst, w)
                        si["on_wait"] = keep
                elif len(waits) > 1:
                    for w in waits[:-1]:
                        hoist(out, inst, w)
                    si["on_wait"] = waits[-1:]
                out.append(inst)
            blk["instructions"] = out
    return bir


def _attach_wait_legalizer(nc):
    import json as _json

    orig_fn = nc.to_json_bytes

    def _patched():
        bir = _json.loads(orig_fn())
        _legalize_waits(bir)
        return _json.dumps(bir).encode()

    nc.to_json_bytes = _patched


def build_module(sp_inv=1.0 / 4096, s2_inv=1.0 / 2048, repeat=1):
    nc = bass.Bass()
    f32 = mybir.dt.float32
    u8 = mybir.dt.uint8

    g8T = nc.dram_tensor("g8T", [GNN, B_LOC], u8, kind="ExternalInput")
    t8T = nc.dram_tensor("t8T", [TR, B_LOC], u8, kind="ExternalInput")
    glT = nc.dram_tensor("glT", [GNN, B_LOC], u8, kind="ExternalInput")
    tlT = nc.dram_tensor("tlT", [TR, B_LOC], u8, kind="ExternalInput")
    p12T = nc.dram_tensor("p12T", [KC * P, F], u8, kind="ExternalInput")
    wghT = nc.dram_tensor("wghT", [GNN, F], u8, kind="ExternalInput")
    wglT = nc.dram_tensor("wglT", [GNN, F], u8, kind="ExternalInput")
    wthT = nc.dram_tensor("wthT", [TR, F], u8, kind="ExternalInput")
    wtlT = nc.dram_tensor("wtlT", [TR, F], u8, kind="ExternalInput")
    w2T = nc.dram_tensor("w2T", [F, F], u8, kind="ExternalInput")
    cv = nc.dram_tensor("cv", [F], f32, kind="ExternalInput")
    bsum = nc.dram_tensor("bsum", [F], f32, kind="ExternalInput")
    outT = nc.dram_tensor("outT", [F, B_LOC], f32, kind="ExternalOutput")

    g8_ap = g8T[:].rearrange("(k p) b -> p k b", p=P)
    t8_ap = t8T[:].rearrange("(k p) b -> p k b", p=P)
    gl_ap = glT[:].rearrange("(k p) b -> p k b", p=P)
    tl_ap = tlT[:].rearrange("(k p) b -> p k b", p=P)
    p12_ap = p12T[:].rearrange("(k p) f -> p k f", p=P)
    out_ap = outT[:].rearrange("(k p) b -> p k b", p=P)

    def wf(dram):  # weight DRAM -> [P, k, F] view
        return dram[:].rearrange("(k p) f -> p k f", p=P)

    with tile.TileContext(nc) as tc:
        with (
            tc.tile_pool(name="const", bufs=1) as const,
            tc.tile_pool(name="io", bufs=IO_BUFS) as io,
            tc.tile_pool(name="hbuf", bufs=2) as hbuf,
            tc.tile_pool(name="psum", bufs=PSUM_BUFS, space="PSUM") as psum,
        ):
            p12 = const.tile([P, KC, F], u8)
            cv_t = const.tile([P, KF], f32)
            wgh = const.tile([P, KG, F], u8)
            wgl = const.tile([P, KG, F], u8)
            wth = const.tile([P, KT, F], u8)
            wtl = const.tile([P, KT, F], u8)
            w2 = const.tile([P, KF, F], u8)
            bs_t = const.tile([P, KF], f32)

            # p12 in three pieces: the C t-sub-phase for the first four output
            # tiles needs only the t-slabs of the first column half
            half = (KF // 2) * P
            nc.sync.dma_start(out=p12[:, 0:KT, 0:half], in_=p12_ap[:, 0:KT, 0:half])

            for blk in [b for _ in range(repeat) for b in range(NBLK)]:
                bs = slice(blk * NB, (blk + 1) * NB)
                t8 = io.tile([P, KT, NB], u8, tag="t8")
                nc.sync.dma_start(out=t8, in_=t8_ap[:, :, bs])
                g8 = io.tile([P, KG, NB], u8, tag="g8")
                nc.sync.dma_start(out=g8, in_=g8_ap[:, :, bs])
                gl = io.tile([P, KG, NB], u8, tag="gl")
                tl = io.tile([P, KT, NB], u8, tag="tl")
                if blk == 0:
                    # stream in D-sub-phase consumption order: the whole first
                    # block stays DMA-fed with no PE stall
                    nc.sync.dma_start(out=p12[:, 0:KT, half:F], in_=p12_ap[:, 0:KT, half:F])
                    nc.sync.dma_start(out=p12[:, KT:KC, :], in_=p12_ap[:, KT:KC, :])
                    nc.sync.dma_start(out=cv_t, in_=cv[:].rearrange("(k p) -> p k", p=P))
                    nc.sync.dma_start(out=wth, in_=wf(wthT))
                    nc.sync.dma_start(out=wgh, in_=wf(wghT))
                    nc.sync.dma_start(out=tl, in_=tl_ap[:, :, bs])
                    nc.sync.dma_start(out=gl, in_=gl_ap[:, :, bs])
                    nc.sync.dma_start(out=wtl, in_=wf(wtlT))
                    nc.sync.dma_start(out=wgl, in_=wf(wglT))
                    nc.sync.dma_start(out=bs_t, in_=bsum[:].rearrange("(k p) -> p k", p=P))
                    nc.sync.dma_start(out=w2, in_=wf(w2T))
                else:
                    nc.sync.dma_start(out=gl, in_=gl_ap[:, :, bs])
                    nc.sync.dma_start(out=tl, in_=tl_ap[:, :, bs])

                h8 = hbuf.tile([P, KF, NB], u8, tag="h8")
                out_t = io.tile([P, KF, NB], f32, tag="out_t")

                def dr(ps, w_t, kofs, j, x_t, kk, start, stop):
                    nc.tensor.matmul(
                        ps,
                        w_t[:, kofs + kk : kofs + kk + 2, ts(j, P)].bitcast(FP8),
                        x_t[:, kk : kk + 2, :].bitcast(FP8),
                        start=start,
                        stop=stop,
                        perf_mode=DR,
                    )

                # C: h = gelu((P1@t + P2@g) * sp_inv + cv)
                # two sub-phases across all 8 PSUM banks: all t-passes first
                # (t8 lands before g8), so the g8 DMA never stalls the PE
                c_ps = []
                for j in range(KF):
                    ps = psum.tile([P, NB], f32, tag="ps")
                    c_ps.append(ps)
                    for kk in range(0, KT, 2):
                        dr(ps, p12, 0, j, t8, kk, kk == 0, False)
                for j in range(KF):
                    ps = c_ps[j]
                    for kk in range(0, KG, 2):
                        dr(ps, p12, KT, j, g8, kk, False, kk + 2 >= KG)
                    nc.scalar.activation(
                        h8[:, j, :].bitcast(FP8), ps, AF.Gelu, bias=cv_t[:, j : j + 1], scale=sp_inv
                    )

                # D: out = (W2@h + Wg@g + Wt@t) * s2_inv + bsum
                # input projections via hi/lo fp8: hi@hi + hi@lo + lo@hi, all
                # at scale s2 in one PSUM group per tile. Sub-phase-major
                # across all 8 banks, ordered to match DMA arrival: hi@hi,
                # then hi@lo (needs gl/tl), then lo@hi (needs wgl/wtl), then
                # W2@h (needs h8) with the epilogue interleaved per tile.
                def d_epilogue(ps, j):
                    nc.scalar.activation(
                        out_t[:, j, :], ps, AF.Identity, bias=bs_t[:, j : j + 1], scale=s2_inv
                    )
                    # out DMA on the Activation HWDGE queue: keeps the SP
                    # input queue free of head-of-line blocking
                    if blk == NBLK - 1:
                        nc.scalar.dma_start(out=out_ap[:, j : j + 1, bs], in_=out_t[:, j : j + 1, :])
                    elif j % 2 == 1:
                        nc.scalar.dma_start(
                            out=out_ap[:, j - 1 : j + 1, bs], in_=out_t[:, j - 1 : j + 1, :]
                        )

                if blk < NBLK - 1:
                    # sub-phase-major across all 8 banks, ordered to match DMA
                    # arrival in block 0 (hi@hi, then hi@lo needing gl/tl,
                    # then lo@hi needing wgl/wtl, then W2@h needing h8)
                    d_ps = []
                    for _j in range(KF):
                        ps = psum.tile([P, NB], f32, tag="ps")
                        d_ps.append(ps)
                    for j in range(KF):
                        for kk in range(0, KT, 2):
                            dr(d_ps[j], wth, 0, j, t8, kk, kk == 0, False)
                        for kk in range(0, KG, 2):
                            dr(d_ps[j], wgh, 0, j, g8, kk, False, False)
                    for j in range(KF):
                        for kk in range(0, KT, 2):
                            dr(d_ps[j], wth, 0, j, tl, kk, False, False)
                        for kk in range(0, KG, 2):
                            dr(d_ps[j], wgh, 0, j, gl, kk, False, False)
                    for j in range(KF):
                        for kk in range(0, KT, 2):
                            dr(d_ps[j], wtl, 0, j, t8, kk, False, False)
                        for kk in range(0, KG, 2):
                            dr(d_ps[j], wgl, 0, j, g8, kk, False, False)
                    for j in range(KF):
                        for kk in range(0, KF, 2):
                            dr(d_ps[j], w2, 0, j, h8, kk, False, kk + 2 >= KF)
                        d_epilogue(d_ps[j], j)
                else:
                    # last block: per-tile groups so epilogues + output DMAs
                    # drain progressively; the final tile is split into 384+128
                    # column groups so the post-last-matmul chain is tiny
                    def d_group(ps, j, c0, c1):
                        def drc(w_t, x_t, kk, start, stop):
                            nc.tensor.matmul(
                                ps[:, 0 : c1 - c0],
                                w_t[:, kk : kk + 2, ts(j, P)].bitcast(FP8),
                                x_t[:, kk : kk + 2, c0:c1].bitcast(FP8),
                                start=start,
                                stop=stop,
                                perf_mode=DR,
                            )

                        for kk in range(0, KT, 2):
                            drc(wth, t8, kk, kk == 0, False)
                        for kk in range(0, KG, 2):
                            drc(wgh, g8, kk, False, False)
                        for kk in range(0, KT, 2):
                            drc(wth, tl, kk, False, False)
                        for kk in range(0, KG, 2):
                            drc(wgh, gl, kk, False, False)
                        for kk in range(0, KT, 2):
                            drc(wtl, t8, kk, False, False)
                        for kk in range(0, KG, 2):
                            drc(wgl, g8, kk, False, False)
                        for kk in range(0, KF, 2):
                            drc(w2, h8, kk, False, kk + 2 >= KF)
                        nc.scalar.activation(
                            out_t[:, j, c0:c1], ps[:, 0 : c1 - c0], AF.Identity,
                            bias=bs_t[:, j : j + 1], scale=s2_inv,
                        )
                        nc.scalar.dma_start(
                            out=out_ap[:, j : j + 1, blk * NB + c0 : blk * NB + c1],
                            in_=out_t[:, j : j + 1, c0:c1],
                        )

                    for j in range(KF):
                        if j < KF - 1:
                            ps = psum.tile([P, NB], f32, tag="ps")
                            d_group(ps, j, 0, NB)
                        else:
                            ps = psum.tile([P, NB], f32, tag="ps")
                            d_group(ps, j, 0, 3 * NB // 4)
                            ps = psum.tile([P, NB], f32, tag="ps")
                            d_group(ps, j, 3 * NB // 4, NB)

    _attach_wait_legalizer(nc)
    return nc


def _pow2_scale(m, target=224.0):
    if m == 0.0 or not np.isfinite(m):
        return 1.0
    return float(2.0 ** np.floor(np.log2(target / m)))


def prepare_inputs(gnn_features, transformer_features, Wg, bg, Wt, bt, Wv, bv, Wo, bo, W1, b1, W2, b2):
    """Host-side: fold attention+fusion prefix down to the raw inputs,
    quantize everything to fp8e4 (hi/lo for the input projections),
    transpose to [feature, batch]."""
    f64 = np.float64
    fp8 = ml_dtypes.float8_e4m3

    A = Wo.astype(f64) @ Wv.astype(f64)
    W1a = W1[:, :F].astype(f64)
    W1b = W1[:, F:].astype(f64)
    M1 = W1a @ A
    M2 = W1b @ A
    c = (W1a + W1b) @ (Wo.astype(f64) @ bv.astype(f64) + bo.astype(f64)) + b1.astype(f64)
    P1 = M1 @ Wt.astype(f64)  # [F, TR]
    P2 = M2 @ Wg.astype(f64)  # [F, GNN]
    cvec = c + M1 @ bt.astype(f64) + M2 @ bg.astype(f64)

    p12 = np.concatenate([P1.T, P2.T], axis=0)  # [TR+GNN, F] rows=contraction
    sp = _pow2_scale(float(np.abs(p12).max()))
    s2 = _pow2_scale(max(float(np.abs(w).max()) for w in (W2, Wg, Wt)))

    def hilo(x):
        hi = x.astype(fp8)
        lo = (x - hi.astype(f64)).astype(fp8)
        return hi.view(np.uint8), lo.view(np.uint8)

    p12T = np.ascontiguousarray(p12 * sp).astype(fp8).view(np.uint8)
    w2T = np.ascontiguousarray(W2.T.astype(f64) * s2).astype(fp8).view(np.uint8)
    wghT, wglT = hilo(np.ascontiguousarray(Wg.T.astype(f64) * s2))
    wthT, wtlT = hilo(np.ascontiguousarray(Wt.T.astype(f64) * s2))
    bsum = (bg.astype(f64) + bt.astype(f64) + b2.astype(f64)).astype(np.float32)

    shared = {
        "p12T": p12T,
        "wghT": wghT,
        "wglT": wglT,
        "wthT": wthT,
        "wtlT": wtlT,
        "w2T": w2T,
        "cv": cvec.astype(np.float32),
        "bsum": bsum,
    }
    in_maps = []
    for i in range(N_CORES):
        rows = slice(i * B_LOC, (i + 1) * B_LOC)
        g8, gl = hilo(np.ascontiguousarray(gnn_features[rows].T).astype(f64))
        t8, tl = hilo(np.ascontiguousarray(transformer_features[rows].T).astype(f64))
        in_maps.append({"g8T": g8, "glT": gl, "t8T": t8, "tlT": tl, **shared})
    return in_maps, 1.0 / sp, 1.0 / s2


def run(inputs, trace=False, **kw):
    in_maps, sp_inv, s2_inv = prepare_inputs(**inputs)
    nc = build_module(sp_inv=sp_inv, s2_inv=s2_inv)
    res = run_bass_kernel_spmd(nc, in_maps, core_ids=list(range(N_CORES)), trace=trace, **kw)
    out = np.concatenate([r["outT"].T for r in res.results], axis=0).astype(np.float32)
    return out, res


def kernel(**inputs) -> np.ndarray:
    out, _ = run(inputs, trace=False)
    return out
